# revision 4
# baseline (speedup 1.0000x reference)
"""ClusterNormZCA Trainium2 kernel (v2: fp8 gram + f16-packed transposes).

Full inputs x[256, 64, 4096] f32 -> Z[256, 64, 4096] f32.
Sharded over batch across 8 NeuronCores (32 batches/core, zero comm).

Per core, batches are processed in pairs ("tiles" of [128, 4096] = 2x64
rows). x is cast once to fp8e4 (with exact f32 row sums via ACT accum);
transposes move two packed fp8 values per PE element by transposing the
float16 reinterpretation (16 transpose matmuls instead of 32); the Gram
matrix is accumulated with fp8 DoubleRow matmuls (256-deep contraction
per instruction at 0.5 cycles/row); Ledoit-Wolf shrinkage stats are
computed with a tiny PE matmul; the inverse square root uses a single
bf16 Newton-Schulz iteration on the trace-normalized shrunk covariance
(spectrum ~= 1, X0 = 1.5I - A/2 is already within ~5e-2); whitening
applies S = I + Delta with Delta and the moving x both in fp8, and the
identity path added in f32. z is stored bf16 (halves output DMA); the
host converts back to f32.
"""

import sys

for _p in ("/opt/trn_rl_repo", "/root/.axon_site/_ro/trn_rl_repo"):
    if _p not in sys.path:
        sys.path.append(_p)

import numpy as np

B, C, M = 256, 64, 4096
N_CORES = 8
B_CORE = B // N_CORES          # 32
NTILES = B_CORE // 2           # 16 pairs per core
NCHUNK = M // 256              # 16 u16 transpose chunks per tile
NGRP = 4                       # transpose chunks per PSUM copyback group
NSLICE = M // 512              # 8 whitening slices per tile
C1 = float(M - 2) / float(M)   # (n-2)/n
C2 = float(M + 2)              # n+2
RINV_M = 1.0 / float(M)

_CACHE = {}


def _consts_np():
    identu = np.eye(128, dtype=np.float16)
    ident = np.eye(128, dtype=np.float32)
    i15 = (1.5 * np.eye(128)).astype(np.float32)
    maskblk = np.zeros((128, 128), dtype=np.float32)
    maskblk[:64, :64] = 1.0
    maskblk[64:, 64:] = 1.0
    bcast = np.zeros((2, 128), dtype=np.float32)
    bcast[0, :64] = 1.0
    bcast[1, 64:] = 1.0
    halves = np.zeros((128, 2), dtype=np.float32)
    halves[:64, 0] = 1.0
    halves[64:, 1] = 1.0
    return {
        "identu": identu,
        "identf": ident,
        "i15": i15,
        "maskblk": maskblk,
        "bcast": bcast,
        "halves": halves,
    }


def _build(ntiles=NTILES):
    import concourse.bacc as bacc
    import concourse.mybir as mybir
    from concourse.tile import TileContext

    f32 = mybir.dt.float32
    bf16 = mybir.dt.bfloat16
    fp8 = mybir.dt.float8e4
    f16 = mybir.dt.float16
    AF = mybir.ActivationFunctionType
    OP = mybir.AluOpType
    AX = mybir.AxisListType
    PM = mybir.MatmulPerfMode

    nc = bacc.Bacc("TRN2", target_bir_lowering=False, debug=False)
    X = nc.declare_dram_parameter("x", [2 * ntiles, C, M], f32, isOutput=False)
    O = nc.declare_dram_parameter("z", [2 * ntiles, C, M], bf16, isOutput=True)
    CONST = {
        "identu": nc.declare_dram_parameter("identu", [128, 128], f16, isOutput=False),
        "identf": nc.declare_dram_parameter("identf", [128, 128], f32, isOutput=False),
        "i15": nc.declare_dram_parameter("i15", [128, 128], f32, isOutput=False),
        "maskblk": nc.declare_dram_parameter("maskblk", [128, 128], f32, isOutput=False),
        "bcast": nc.declare_dram_parameter("bcast", [2, 128], f32, isOutput=False),
        "halves": nc.declare_dram_parameter("halves", [128, 2], f32, isOutput=False),
    }

    with TileContext(nc) as tc:
        with (
            tc.tile_pool(name="cpool", bufs=1) as cpool,
            tc.tile_pool(name="xin", bufs=3) as xin_p,
            tc.tile_pool(name="x8p", bufs=3) as x8_p,
            tc.tile_pool(name="ybuf", bufs=2) as ybuf_p,
            tc.tile_pool(name="zout", bufs=2) as zout_p,
            tc.tile_pool(name="mid", bufs=2) as mid_p,
            tc.tile_pool(name="tiny", bufs=2) as tiny_p,
            tc.tile_pool(name="tvp", bufs=2) as tvp_p,
            tc.tile_pool(name="tps", bufs=2, space="PSUM") as tps_p,
            tc.tile_pool(name="gps", bufs=2, space="PSUM") as gps_p,
            tc.tile_pool(name="nsp", bufs=2, space="PSUM") as nsp_p,
            tc.tile_pool(name="wps", bufs=2, space="PSUM") as wps_p,
        ):
            cb = {}
            for nm, hd in CONST.items():
                shp = list(hd.shape)
                dt = hd.dtype
                t = cpool.tile(shp, dt, name=f"c_{nm}")
                nc.sync.dma_start(out=t, in_=hd[:])
                cb[nm] = t
            identu, identf = cb["identu"], cb["identf"]
            i15, maskblk = cb["i15"], cb["maskblk"]
            bcast, halves = cb["bcast"], cb["halves"]

            for t in range(ntiles):
                # ---- load pair of batches ----
                xt = xin_p.tile([128, M], f32, name="xt")
                nc.sync.dma_start(
                    out=xt, in_=X[2 * t : 2 * t + 2].rearrange("b c m -> (b c) m")
                )

                # ---- cast to fp8 + fp32 row sums (ACT) ----
                x8 = x8_p.tile([128, M], fp8, name="x8")
                sacc = tiny_p.tile([128, 1], f32, name="sacc")
                nc.scalar.activation(x8, xt, AF.Copy, accum_out=sacc)
                xu = x8.bitcast(f16)  # [128, 2048], each elem = 2 packed fp8

                # ---- transposes of f16-packed pairs (PE) + copyback ----
                # Each f16 transpose moves 2 packed fp8 values; the copyback
                # deinterleaves the (c, s) pairs into per-group s-planes so
                # the DoubleRow gram weights AP has plane stride 512 (the
                # ISA requires the 2-plane dim step to be a multiple of 16).
                # ybuf fp8 layout, group g (4 chunks, m in [1024g,1024g+1024)):
                #   [1024g + 512s + 128j + c] = x[c, 1024g + 256j + 2p + s]
                ybuf = ybuf_p.tile([128, 4096], fp8, name="ybuf")
                for g in range(NCHUNK // NGRP):
                    tps = tps_p.tile([128, NGRP * 128], f16, name="tps")
                    for j in range(NGRP):
                        k = NGRP * g + j
                        nc.tensor.transpose(
                            tps[:, 128 * j : 128 * (j + 1)],
                            xu[:, 128 * k : 128 * (k + 1)],
                            identu,
                        )
                    tp2 = tps.bitcast(fp8).rearrange("p (q two) -> p two q", two=2)
                    for s in range(2):
                        dst = ybuf[:, 1024 * g + 512 * s : 1024 * g + 512 * (s + 1)]
                        if (2 * g + s) % 2 == 0:
                            nc.scalar.copy(dst, tp2[:, s, :])
                        else:
                            nc.vector.tensor_copy(dst, tp2[:, s, :])

                # ---- Gram accumulation (PE, fp8 DoubleRow) ----
                gps = gps_p.tile([128, 128], f32, name="gps")
                for g in range(NCHUNK // NGRP):
                    yg = ybuf[:, 1024 * g : 1024 * (g + 1)].rearrange(
                        "p (two q) -> p two q", two=2
                    )
                    for j in range(NGRP):
                        yk = yg[:, :, 128 * j : 128 * (j + 1)]
                        nc.tensor.matmul(
                            gps, yk, yk, start=(g == 0 and j == 0), stop=False,
                            perf_mode=PM.DoubleRow,
                        )

                # rank-1 mean correction: G -= s s^T / M (cross blocks
                # polluted, but masked out downstream)
                sml = nsp_p.tile([128, 512], f32, name="sml", tag="nsp")
                srp = sml[0:1, 384:512]
                nc.tensor.transpose(srp, sacc, identf)
                sneg = tiny_p.tile([1, 128], bf16, name="sneg")
                nc.scalar.activation(sneg, srp, AF.Identity, scale=-RINV_M)
                s16 = tiny_p.tile([1, 128], bf16, name="s16")
                nc.scalar.copy(s16, srp)
                nc.tensor.matmul(gps, sneg, s16, start=False, stop=True)

                # ---- shrinkage stats ----
                mg = mid_p.tile([128, 128], f32, name="mg")
                nc.vector.tensor_tensor(out=mg, in0=gps, in1=maskblk, op=OP.mult)
                dtmp = mid_p.tile([128, 128], f32, name="dtmp")
                nc.gpsimd.tensor_tensor(out=dtmp, in0=mg, in1=identf, op=OP.mult)
                statc = tiny_p.tile([128, 2], f32, name="statc")
                nc.vector.tensor_reduce(
                    out=statc[:, 0:1], in_=dtmp, axis=AX.X, op=OP.add
                )
                sqt = mid_p.tile([128, 128], f32, name="sqt")
                nc.gpsimd.tensor_tensor(out=sqt, in0=mg, in1=mg, op=OP.mult)
                nc.vector.tensor_reduce(
                    out=statc[:, 1:2], in_=sqt, axis=AX.X, op=OP.add
                )
                # [2,2]: row h = (D, SQ) of batch h
                stp = sml[0:2, 256:258]
                nc.tensor.matmul(stp, halves, statc, start=True, stop=True)
                st = tiny_p.tile([2, 2], f32, name="st")
                nc.vector.tensor_copy(st, stp)

                # rho chain on [2,1]
                D = st[:, 0:1]
                SQ = st[:, 1:2]
                dsq = tiny_p.tile([2, 8], f32, name="dsq")
                nc.vector.tensor_tensor(out=dsq[:, 0:1], in0=D, in1=D, op=OP.mult)
                nc.vector.scalar_tensor_tensor(
                    out=dsq[:, 1:2], in0=SQ, scalar=C1, in1=dsq[:, 0:1],
                    op0=OP.mult, op1=OP.add,
                )  # num
                nc.vector.scalar_tensor_tensor(
                    out=dsq[:, 2:3], in0=dsq[:, 0:1], scalar=-1.0 / 64.0,
                    in1=SQ, op0=OP.mult, op1=OP.add,
                )  # den0
                nc.vector.reciprocal(dsq[:, 3:4], dsq[:, 2:3])
                nc.vector.tensor_tensor(
                    out=dsq[:, 4:5], in0=dsq[:, 1:2], in1=dsq[:, 3:4], op=OP.mult
                )
                scl3 = tiny_p.tile([2, 3], f32, name="scl3")
                nc.vector.tensor_scalar(
                    out=scl3[:, 1:2], in0=dsq[:, 4:5], scalar1=1.0 / C2,
                    op0=OP.mult, scalar2=1.0, op1=OP.min,
                )  # rho
                nc.vector.tensor_scalar(
                    out=dsq[:, 5:6], in0=scl3[:, 1:2], scalar1=-64.0,
                    op0=OP.mult, scalar2=64.0, op1=OP.add,
                )  # 64(1-rho)
                nc.vector.reciprocal(dsq[:, 6:7], D)
                nc.vector.tensor_tensor(
                    out=scl3[:, 0:1], in0=dsq[:, 5:6], in1=dsq[:, 6:7], op=OP.mult
                )  # s1 = 64(1-rho)/D
                nc.scalar.sqrt(dsq[:, 7:8], dsq[:, 6:7])
                nc.scalar.mul(scl3[:, 2:3], dsq[:, 7:8], 512.0)  # rsc = 512/sqrt(D)

                # broadcast (s1, rho, rsc) to [128,3]
                bps = sml[:, 260:263]
                nc.tensor.matmul(bps, bcast, scl3, start=True, stop=True)
                bcols = tiny_p.tile([128, 3], f32, name="bcols")
                nc.vector.tensor_copy(bcols, bps)
                s1v = bcols[:, 0:1]
                rhov = bcols[:, 1:2]
                rscv = bcols[:, 2:3]

                # ---- Ahat = s1*mg + rho*I ; X0 = 1.5I - 0.5*Ahat (bf16) ----
                irho = mid_p.tile([128, 128], f32, name="irho")
                nc.scalar.activation(irho, identf, AF.Identity, scale=rhov)
                ahat = mid_p.tile([128, 128], bf16, name="ahat")
                nc.vector.scalar_tensor_tensor(
                    out=ahat, in0=mg, scalar=s1v, in1=irho, op0=OP.mult, op1=OP.add
                )
                xcur = mid_p.tile([128, 128], bf16, name="xcur")
                nc.vector.scalar_tensor_tensor(
                    out=xcur, in0=ahat, scalar=-0.5, in1=i15, op0=OP.mult, op1=OP.add
                )

                # ---- single bf16 Newton-Schulz iteration ----
                p1 = sml[:, 0:128]
                nc.tensor.matmul(p1, xcur, xcur, start=True, stop=True)
                x2 = mid_p.tile([128, 128], bf16, name="x2")
                nc.scalar.copy(x2, p1)
                p2 = sml[:, 128:256]
                nc.tensor.matmul(p2, ahat, x2, start=True, stop=True)
                u = mid_p.tile([128, 128], bf16, name="u")
                nc.vector.scalar_tensor_tensor(
                    out=u, in0=p2, scalar=-0.5, in1=i15, op0=OP.mult, op1=OP.add
                )
                p3 = sml[:, 0:128]
                nc.tensor.matmul(p3, xcur, u, start=True, stop=True)
                # S (bf16, incl. 512/sqrt(D) denormalization)
                ssb = mid_p.tile([128, 128], bf16, name="ssb")
                nc.scalar.activation(ssb, p3, AF.Identity, scale=rscv)
                delta = mid_p.tile([128, 128], fp8, name="delta")
                nc.gpsimd.tensor_tensor(out=delta, in0=ssb, in1=identf, op=OP.subtract)

                # v = S @ mu ; negv = -v
                mu = tiny_p.tile([128, 1], bf16, name="mu")
                nc.scalar.mul(mu, sacc, RINV_M)
                vps = sml[:, 280:281]
                nc.tensor.matmul(vps, ssb, mu, start=True, stop=True)
                negv = tiny_p.tile([128, 1], f32, name="negv")
                nc.scalar.activation(negv, vps, AF.Identity, scale=-1.0)

                # ---- whitening + fused output (bf16 z) ----
                zt = zout_p.tile([128, M], bf16, name="zt")
                for s in range(NSLICE):
                    sl = slice(512 * s, 512 * (s + 1))
                    wps = wps_p.tile([128, 512], f32, name="wps", tag="wps")
                    nc.tensor.matmul(wps, delta, x8[:, sl], start=True, stop=True)
                    if s % 3 == 2:
                        tv = tvp_p.tile([128, 512], f32, name="tv")
                        nc.scalar.activation(
                            tv, wps, AF.Identity, bias=negv[:, 0:1], scale=1.0
                        )
                        nc.gpsimd.tensor_tensor(
                            out=zt[:, sl], in0=tv, in1=xt[:, sl], op=OP.add
                        )
                    else:
                        nc.vector.scalar_tensor_tensor(
                            out=zt[:, sl], in0=wps, scalar=negv[:, 0:1],
                            in1=xt[:, sl], op0=OP.add, op1=OP.add,
                        )
                nc.sync.dma_start(
                    out=O[2 * t : 2 * t + 2].rearrange("b c m -> (b c) m"), in_=zt
                )

    nc.compile()
    return nc


def _get_nc(ntiles=NTILES):
    key = ("nc", ntiles)
    if key not in _CACHE:
        _CACHE[key] = _build(ntiles)
    return _CACHE[key]


def _install_ntff_hook():
    """Provide antenv.axon_hooks (absent in this image) so
    run_bass_kernel_spmd(trace=True) can capture NTFF profiles."""
    import types

    import antenv

    if "antenv.axon_hooks" in sys.modules:
        return
    mod = types.ModuleType("antenv.axon_hooks")
    state = [None]
    mod.set_axon_ntff_profile_hook = lambda h: state.__setitem__(0, h)
    mod.get_axon_ntff_profile_hook = lambda: state[0]
    sys.modules["antenv.axon_hooks"] = mod
    antenv.axon_hooks = mod
    try:
        from trn_agent_boot.trn_boot import _ntff_profile_via_ctypes

        mod.set_axon_ntff_profile_hook(
            _ntff_profile_via_ctypes("/opt/axon/libaxon_pjrt.so")
        )
    except Exception:
        pass


def _run(x, trace=False):
    from concourse.bass_utils import run_bass_kernel_spmd

    if trace:
        _install_ntff_hook()

    nc = _get_nc()
    consts = _consts_np()
    x = np.ascontiguousarray(x, dtype=np.float32)
    in_maps = [
        {"x": x[i * B_CORE : (i + 1) * B_CORE], **consts} for i in range(N_CORES)
    ]
    res = run_bass_kernel_spmd(
        nc, in_maps, list(range(N_CORES)), trace=trace
    )
    out = np.concatenate(
        [res.results[i]["z"].astype(np.float32) for i in range(N_CORES)], axis=0
    )
    return out, res


def kernel(x):
    out, _ = _run(x)
    return out


# revision 5
# speedup vs baseline: 1.2088x; 1.2088x over previous
"""ClusterNormZCA Trainium2 kernel (v2: fp8 gram + f16-packed transposes).

Full inputs x[256, 64, 4096] f32 -> Z[256, 64, 4096] f32.
Sharded over batch across 8 NeuronCores (32 batches/core, zero comm).

Per core, batches are processed in pairs ("tiles" of [128, 4096] = 2x64
rows). x is cast once to fp8e4 (with exact f32 row sums via ACT accum);
transposes move two packed fp8 values per PE element by transposing the
float16 reinterpretation (16 transpose matmuls instead of 32); the Gram
matrix is accumulated with fp8 DoubleRow matmuls (256-deep contraction
per instruction at 0.5 cycles/row); Ledoit-Wolf shrinkage stats are
computed with a tiny PE matmul; the inverse square root uses a single
bf16 Newton-Schulz iteration on the trace-normalized shrunk covariance
(spectrum ~= 1, X0 = 1.5I - A/2 is already within ~5e-2); whitening
applies S = I + Delta with Delta and the moving x both in fp8, and the
identity path added in f32. z is stored bf16 (halves output DMA); the
host converts back to f32.
"""

import sys

for _p in ("/opt/trn_rl_repo", "/root/.axon_site/_ro/trn_rl_repo"):
    if _p not in sys.path:
        sys.path.append(_p)

import numpy as np

B, C, M = 256, 64, 4096
N_CORES = 8
B_CORE = B // N_CORES          # 32
NTILES = B_CORE // 2           # 16 pairs per core
NCHUNK = M // 256              # 16 u16 transpose chunks per tile
NGRP = 4                       # transpose chunks per PSUM copyback group
NSLICE = M // 512              # 8 whitening slices per tile
C1 = float(M - 2) / float(M)   # (n-2)/n
C2 = float(M + 2)              # n+2
RINV_M = 1.0 / float(M)

_CACHE = {}


def _consts_np():
    identu = np.eye(128, dtype=np.float16)
    ident = np.eye(128, dtype=np.float32)
    i15 = (1.5 * np.eye(128)).astype(np.float32)
    maskblk = np.zeros((128, 128), dtype=np.float32)
    maskblk[:64, :64] = 1.0
    maskblk[64:, 64:] = 1.0
    bcast = np.zeros((2, 128), dtype=np.float32)
    bcast[0, :64] = 1.0
    bcast[1, 64:] = 1.0
    halves = np.zeros((128, 2), dtype=np.float32)
    halves[:64, 0] = 1.0
    halves[64:, 1] = 1.0
    return {
        "identu": identu,
        "identf": ident,
        "i15": i15,
        "maskblk": maskblk,
        "bcast": bcast,
        "halves": halves,
    }


def _build(ntiles=NTILES):
    import concourse.bacc as bacc
    import concourse.mybir as mybir
    from concourse.tile import TileContext

    f32 = mybir.dt.float32
    bf16 = mybir.dt.bfloat16
    fp8 = mybir.dt.float8e4
    f16 = mybir.dt.float16
    AF = mybir.ActivationFunctionType
    OP = mybir.AluOpType
    AX = mybir.AxisListType
    PM = mybir.MatmulPerfMode

    nc = bacc.Bacc("TRN2", target_bir_lowering=False, debug=False)
    X = nc.declare_dram_parameter("x", [2 * ntiles, C, M], f32, isOutput=False)
    O = nc.declare_dram_parameter("z", [2 * ntiles, C, M], bf16, isOutput=True)
    CONST = {
        "identu": nc.declare_dram_parameter("identu", [128, 128], f16, isOutput=False),
        "identf": nc.declare_dram_parameter("identf", [128, 128], f32, isOutput=False),
        "i15": nc.declare_dram_parameter("i15", [128, 128], f32, isOutput=False),
        "maskblk": nc.declare_dram_parameter("maskblk", [128, 128], f32, isOutput=False),
        "bcast": nc.declare_dram_parameter("bcast", [2, 128], f32, isOutput=False),
        "halves": nc.declare_dram_parameter("halves", [128, 2], f32, isOutput=False),
    }

    with TileContext(nc) as tc:
        with (
            tc.tile_pool(name="cpool", bufs=1) as cpool,
            tc.tile_pool(name="xin", bufs=3) as xin_p,
            tc.tile_pool(name="x8p", bufs=3) as x8_p,
            tc.tile_pool(name="ybuf", bufs=2) as ybuf_p,
            tc.tile_pool(name="zout", bufs=2) as zout_p,
            tc.tile_pool(name="mid", bufs=2) as mid_p,
            tc.tile_pool(name="tiny", bufs=2) as tiny_p,
            tc.tile_pool(name="tvp", bufs=2) as tvp_p,
            tc.tile_pool(name="tps", bufs=2, space="PSUM") as tps_p,
            tc.tile_pool(name="gps", bufs=2, space="PSUM") as gps_p,
            tc.tile_pool(name="nsp", bufs=2, space="PSUM") as nsp_p,
            tc.tile_pool(name="wps", bufs=2, space="PSUM") as wps_p,
        ):
            cb = {}
            for nm, hd in CONST.items():
                shp = list(hd.shape)
                dt = hd.dtype
                t = cpool.tile(shp, dt, name=f"c_{nm}")
                nc.sync.dma_start(out=t, in_=hd[:])
                cb[nm] = t
            identu, identf = cb["identu"], cb["identf"]
            i15, maskblk = cb["i15"], cb["maskblk"]
            bcast, halves = cb["bcast"], cb["halves"]

            def front(t):
                """Load/cast/transpose/gram for tile t; returns live state."""
                # ---- load pair of batches (4 m-slices for early cast) ----
                xt = xin_p.tile([128, M], f32, name="xt")
                xdr = X[2 * t : 2 * t + 2].rearrange("b c m -> (b c) m")
                for q in range(4):
                    sl = slice(1024 * q, 1024 * (q + 1))
                    nc.sync.dma_start(out=xt[:, sl], in_=xdr[:, sl])

                # ---- cast to fp8 + fp32 row sums (ACT, 4 slices) ----
                x8 = x8_p.tile([128, M], fp8, name="x8")
                sac4 = tiny_p.tile([128, 4], f32, name="sac4")
                for q in range(4):
                    sl = slice(1024 * q, 1024 * (q + 1))
                    nc.scalar.activation(
                        x8[:, sl], xt[:, sl], AF.Copy, accum_out=sac4[:, q : q + 1]
                    )
                sacc = tiny_p.tile([128, 1], f32, name="sacc")
                nc.vector.tensor_reduce(out=sacc, in_=sac4, axis=AX.X, op=OP.add)
                xu = x8.bitcast(f16)  # [128, 2048], each elem = 2 packed fp8

                # ---- transposes of f16-packed pairs (PE) + copyback ----
                # Each f16 transpose moves 2 packed fp8 values; the copyback
                # deinterleaves the (c, s) pairs into per-group s-planes so
                # the DoubleRow gram weights AP has plane stride 512 (the
                # ISA requires the 2-plane dim step to be a multiple of 16).
                # ybuf fp8 layout, group g (4 chunks, m in [1024g,1024g+1024)):
                #   [1024g + 512s + 128j + c] = x[c, 1024g + 256j + 2p + s]
                ybuf = ybuf_p.tile([128, 4096], fp8, name="ybuf")
                gps = gps_p.tile([128, 128], f32, name="gps")
                for g in range(NCHUNK // NGRP):
                    tps = tps_p.tile([128, NGRP * 128], f16, name="tps")
                    for j in range(NGRP):
                        k = NGRP * g + j
                        nc.tensor.transpose(
                            tps[:, 128 * j : 128 * (j + 1)],
                            xu[:, 128 * k : 128 * (k + 1)],
                            identu,
                        )
                    tp2 = tps.bitcast(fp8).rearrange("p (q two) -> p two q", two=2)
                    for s in range(2):
                        dst = ybuf[:, 1024 * g + 512 * s : 1024 * g + 512 * (s + 1)]
                        if s == 0:
                            nc.scalar.copy(dst, tp2[:, s, :])
                        else:
                            nc.vector.tensor_copy(dst, tp2[:, s, :])
                    # ---- Gram accumulation (PE, fp8 DoubleRow) ----
                    # issued right after this group's copyback; group g's
                    # matmuls overlap group g+1's transposes
                    yg = ybuf[:, 1024 * g : 1024 * (g + 1)].rearrange(
                        "p (two q) -> p two q", two=2
                    )
                    for j in range(NGRP):
                        yk = yg[:, :, 128 * j : 128 * (j + 1)]
                        nc.tensor.matmul(
                            gps, yk, yk, start=(g == 0 and j == 0), stop=False,
                            perf_mode=PM.DoubleRow,
                        )

                # rank-1 mean correction: G -= s s^T / M (cross blocks
                # polluted, but masked out downstream)
                sml = nsp_p.tile([128, 512], f32, name="sml", tag="nsp")
                srp = sml[0:1, 384:512]
                nc.tensor.transpose(srp, sacc, identf)
                sneg = tiny_p.tile([1, 128], bf16, name="sneg")
                nc.scalar.activation(sneg, srp, AF.Identity, scale=-RINV_M)
                s16 = tiny_p.tile([1, 128], bf16, name="s16")
                nc.scalar.copy(s16, srp)
                nc.tensor.matmul(gps, sneg, s16, start=False, stop=True)
                return (t, xt, x8, sacc, gps, sml)

            def back(state):
                """Stats/NS/whitening/store for a tile whose front is done."""
                t, xt, x8, sacc, gps, sml = state
                # ---- shrinkage stats ----
                mg = mid_p.tile([128, 128], f32, name="mg")
                nc.vector.tensor_tensor(out=mg, in0=gps, in1=maskblk, op=OP.mult)
                dtmp = mid_p.tile([128, 128], f32, name="dtmp")
                nc.gpsimd.tensor_tensor(out=dtmp, in0=mg, in1=identf, op=OP.mult)
                statc = tiny_p.tile([128, 2], f32, name="statc")
                nc.vector.tensor_reduce(
                    out=statc[:, 0:1], in_=dtmp, axis=AX.X, op=OP.add
                )
                sqt = mid_p.tile([128, 128], f32, name="sqt")
                nc.gpsimd.tensor_tensor(out=sqt, in0=mg, in1=mg, op=OP.mult)
                nc.vector.tensor_reduce(
                    out=statc[:, 1:2], in_=sqt, axis=AX.X, op=OP.add
                )
                # [2,2]: row h = (D, SQ) of batch h
                stp = sml[0:2, 256:258]
                nc.tensor.matmul(stp, halves, statc, start=True, stop=True)
                st = tiny_p.tile([2, 2], f32, name="st")
                nc.vector.tensor_copy(st, stp)

                # rho chain on [2,1]
                D = st[:, 0:1]
                SQ = st[:, 1:2]
                dsq = tiny_p.tile([2, 8], f32, name="dsq")
                nc.vector.tensor_tensor(out=dsq[:, 0:1], in0=D, in1=D, op=OP.mult)
                nc.vector.scalar_tensor_tensor(
                    out=dsq[:, 1:2], in0=SQ, scalar=C1, in1=dsq[:, 0:1],
                    op0=OP.mult, op1=OP.add,
                )  # num
                nc.vector.scalar_tensor_tensor(
                    out=dsq[:, 2:3], in0=dsq[:, 0:1], scalar=-1.0 / 64.0,
                    in1=SQ, op0=OP.mult, op1=OP.add,
                )  # den0
                nc.vector.reciprocal(dsq[:, 3:4], dsq[:, 2:3])
                nc.vector.tensor_tensor(
                    out=dsq[:, 4:5], in0=dsq[:, 1:2], in1=dsq[:, 3:4], op=OP.mult
                )
                scl3 = tiny_p.tile([2, 3], f32, name="scl3")
                nc.vector.tensor_scalar(
                    out=scl3[:, 1:2], in0=dsq[:, 4:5], scalar1=1.0 / C2,
                    op0=OP.mult, scalar2=1.0, op1=OP.min,
                )  # rho
                nc.vector.tensor_scalar(
                    out=dsq[:, 5:6], in0=scl3[:, 1:2], scalar1=-64.0,
                    op0=OP.mult, scalar2=64.0, op1=OP.add,
                )  # 64(1-rho)
                nc.vector.reciprocal(dsq[:, 6:7], D)
                nc.vector.tensor_tensor(
                    out=scl3[:, 0:1], in0=dsq[:, 5:6], in1=dsq[:, 6:7], op=OP.mult
                )  # s1 = 64(1-rho)/D
                nc.scalar.sqrt(dsq[:, 7:8], dsq[:, 6:7])
                nc.scalar.mul(scl3[:, 2:3], dsq[:, 7:8], 512.0)  # rsc = 512/sqrt(D)

                # broadcast (s1, rho, rsc) to [128,3]
                bps = sml[:, 260:263]
                nc.tensor.matmul(bps, bcast, scl3, start=True, stop=True)
                bcols = tiny_p.tile([128, 3], f32, name="bcols")
                nc.vector.tensor_copy(bcols, bps)
                s1v = bcols[:, 0:1]
                rhov = bcols[:, 1:2]
                rscv = bcols[:, 2:3]

                # ---- Ahat = s1*mg + rho*I ; X0 = 1.5I - 0.5*Ahat (bf16) ----
                irho = mid_p.tile([128, 128], f32, name="irho")
                nc.scalar.activation(irho, identf, AF.Identity, scale=rhov)
                ahat = mid_p.tile([128, 128], bf16, name="ahat")
                nc.vector.scalar_tensor_tensor(
                    out=ahat, in0=mg, scalar=s1v, in1=irho, op0=OP.mult, op1=OP.add
                )
                xcur = mid_p.tile([128, 128], bf16, name="xcur")
                nc.vector.scalar_tensor_tensor(
                    out=xcur, in0=ahat, scalar=-0.5, in1=i15, op0=OP.mult, op1=OP.add
                )

                # ---- single bf16 Newton-Schulz iteration ----
                p1 = sml[:, 0:128]
                nc.tensor.matmul(p1, xcur, xcur, start=True, stop=True)
                x2 = mid_p.tile([128, 128], bf16, name="x2")
                nc.scalar.copy(x2, p1)
                p2 = sml[:, 128:256]
                nc.tensor.matmul(p2, ahat, x2, start=True, stop=True)
                u = mid_p.tile([128, 128], bf16, name="u")
                nc.vector.scalar_tensor_tensor(
                    out=u, in0=p2, scalar=-0.5, in1=i15, op0=OP.mult, op1=OP.add
                )
                p3 = sml[:, 0:128]
                nc.tensor.matmul(p3, xcur, u, start=True, stop=True)
                # S (bf16, incl. 512/sqrt(D) denormalization)
                ssb = mid_p.tile([128, 128], bf16, name="ssb")
                nc.scalar.activation(ssb, p3, AF.Identity, scale=rscv)
                delta = mid_p.tile([128, 128], fp8, name="delta")
                nc.gpsimd.tensor_tensor(out=delta, in0=ssb, in1=identf, op=OP.subtract)

                # v = S @ mu ; negv = -v
                mu = tiny_p.tile([128, 1], bf16, name="mu")
                nc.scalar.mul(mu, sacc, RINV_M)
                vps = sml[:, 280:281]
                nc.tensor.matmul(vps, ssb, mu, start=True, stop=True)
                negv = tiny_p.tile([128, 1], f32, name="negv")
                nc.scalar.activation(negv, vps, AF.Identity, scale=-1.0)

                # ---- whitening + fused output (bf16 z) ----
                zt = zout_p.tile([128, M], bf16, name="zt")
                for s in range(NSLICE):
                    sl = slice(512 * s, 512 * (s + 1))
                    wps = wps_p.tile([128, 512], f32, name="wps", tag="wps")
                    nc.tensor.matmul(wps, delta, x8[:, sl], start=True, stop=True)
                    if s % 3 == 2:
                        tv = tvp_p.tile([128, 512], f32, name="tv")
                        nc.scalar.activation(
                            tv, wps, AF.Identity, bias=negv[:, 0:1], scale=1.0
                        )
                        nc.gpsimd.tensor_tensor(
                            out=zt[:, sl], in0=tv, in1=xt[:, sl], op=OP.add
                        )
                    else:
                        nc.vector.scalar_tensor_tensor(
                            out=zt[:, sl], in0=wps, scalar=negv[:, 0:1],
                            in1=xt[:, sl], op0=OP.add, op1=OP.add,
                        )
                nc.sync.dma_start(
                    out=O[2 * t : 2 * t + 2].rearrange("b c m -> (b c) m"), in_=zt
                )

            # two-stage software pipeline: front(t) runs while back(t-1)
            # drains, hiding the cross-engine stats/NS latency under the
            # next tile's PE transpose/gram phase.
            state = None
            for t in range(ntiles + 1):
                nxt = front(t) if t < ntiles else None
                if state is not None:
                    back(state)
                state = nxt

    nc.compile()
    return nc


def _get_nc(ntiles=NTILES):
    key = ("nc", ntiles)
    if key not in _CACHE:
        _CACHE[key] = _build(ntiles)
    return _CACHE[key]


def _install_ntff_hook():
    """Provide antenv.axon_hooks (absent in this image) so
    run_bass_kernel_spmd(trace=True) can capture NTFF profiles."""
    import types

    import antenv

    if "antenv.axon_hooks" in sys.modules:
        return
    mod = types.ModuleType("antenv.axon_hooks")
    state = [None]
    mod.set_axon_ntff_profile_hook = lambda h: state.__setitem__(0, h)
    mod.get_axon_ntff_profile_hook = lambda: state[0]
    sys.modules["antenv.axon_hooks"] = mod
    antenv.axon_hooks = mod
    try:
        from trn_agent_boot.trn_boot import _ntff_profile_via_ctypes

        mod.set_axon_ntff_profile_hook(
            _ntff_profile_via_ctypes("/opt/axon/libaxon_pjrt.so")
        )
    except Exception:
        pass


def _run(x, trace=False):
    from concourse.bass_utils import run_bass_kernel_spmd

    if trace:
        _install_ntff_hook()

    nc = _get_nc()
    consts = _consts_np()
    x = np.ascontiguousarray(x, dtype=np.float32)
    in_maps = [
        {"x": x[i * B_CORE : (i + 1) * B_CORE], **consts} for i in range(N_CORES)
    ]
    res = run_bass_kernel_spmd(
        nc, in_maps, list(range(N_CORES)), trace=trace
    )
    out = np.concatenate(
        [res.results[i]["z"].astype(np.float32) for i in range(N_CORES)], axis=0
    )
    return out, res


def kernel(x):
    out, _ = _run(x)
    return out
